# revision 7
# baseline (speedup 1.0000x reference)
"""MoLoRA (top-2 MoE LoRA routing) Trainium2 kernel — fp16 data-path version.

Full inputs -> shard tokens across 8 NeuronCores -> Bass/Tile kernel per core
-> gather full output.

Math (per token):
  logits = silu(x @ W1 + b1) @ W2 + b2
  top-2 renormalized softmax weights: w_top1 = sigmoid(l1 - l2),
  w_top2 = 1 - w_top1 (exact identity — no exp/renorm needed).
  combined = sum_e w_e * (x @ A_e @ B_e) * 2.0 ; out = base + combined.

Kernel strategy per core (2048 tokens, fp16 on the wire):
  - x is pre-transposed on the HOST into xT tiles [128 D-part, KD, TT] so no
    on-chip transposes are needed; contractions over D run at full PE rate.
  - xT tiles stream in 4 chunks so mm1 starts ~2us after the W1 load lands.
  - Router mm1 in token-on-free layout; logits are produced directly
    token-major by a second matmul with hs chunks as the stationary operand.
  - Top-2 weights via max / masked-second-max / sigmoid(delta).
  - Selected-expert weights expand to the stacked expert-rank dim [80] with a
    0/1 matmul; lowT = A_all^T @ xT is scaled by them; combined output is
    lsc^T @ B_all (B pre-scaled by 2.0 on host) + base, written as fp16 and
    widened to fp32 on the host. The +base epilogue alternates between DVE
    (direct PSUM add) and ACT-copy + Pool-add to balance engine load.
"""
import sys

for _p in ("/opt/trn_rl_repo",):
    if _p not in sys.path:
        sys.path.insert(0, _p)

import numpy as np
from contextlib import ExitStack

import concourse.bass as bass
import concourse.tile as tile
from concourse import bacc, mybir
from concourse.bass_utils import run_bass_kernel_spmd

FP = mybir.dt.float32
F16 = mybir.dt.float16
NEG_BIG = -1e30

N_CORES = 8
B_, S, D = 4, 4096, 2048
E, R, H = 5, 16, 256
SCALING = 32.0 / 16.0
TT = 512
TOK = (B_ * S) // N_CORES


def _build_nc(TOK=TOK, D=D, H=H, E=E, R=R, TT=TT, n_cores=N_CORES):
    from concourse.alu_op_type import AluOpType as A

    NCH = TT // 128
    KD = D // 128
    KH = H // 128
    NT = TOK // TT
    M = E * R
    ND = D // 512
    XC = 4          # xt load chunks per tile
    KC = KD // XC   # k-blocks per chunk

    nc = bacc.Bacc("TRN2", num_devices=n_cores, debug=False)

    xt_d = nc.dram_tensor("xt", [NT * 128, KD * TT], F16, kind="ExternalInput")
    base_d = nc.dram_tensor("base", [NT * 128, NCH * D], F16, kind="ExternalInput")
    a_d = nc.dram_tensor("a_all", [128, KD * M], F16, kind="ExternalInput")
    b_d = nc.dram_tensor("b_all", [M, D], F16, kind="ExternalInput")
    w1_d = nc.dram_tensor("w1", [128, KD * H], F16, kind="ExternalInput")
    # packed small constants: f32 [128, KH + NCH*E] = b1 | b2-broadcast
    sm32_d = nc.dram_tensor("sm32", [128, KH + NCH * E], FP, kind="ExternalInput")
    # packed small constants: f16 [128, KH*E + M + 128] = w2 | e80 | ident
    sm16_d = nc.dram_tensor(
        "sm16", [128, KH * E + M + 128], F16, kind="ExternalInput"
    )
    out_d = nc.dram_tensor("out", [NT * 128, NCH * D], F16, kind="ExternalOutput")

    with tile.TileContext(nc) as tc, ExitStack() as ctx:
        const = ctx.enter_context(tc.tile_pool(name="const", bufs=1))
        xt_pool = ctx.enter_context(tc.tile_pool(name="xt", bufs=3))
        base_pool = ctx.enter_context(tc.tile_pool(name="basep", bufs=2))
        out_pool = ctx.enter_context(tc.tile_pool(name="outp", bufs=6))
        cmb_pool = ctx.enter_context(tc.tile_pool(name="cmb", bufs=4))
        zs_pool = ctx.enter_context(tc.tile_pool(name="zs", bufs=2))
        sm_pool = ctx.enter_context(tc.tile_pool(name="sm", bufs=2))
        lsc_pool = ctx.enter_context(tc.tile_pool(name="lsc", bufs=2))

        ps_h = ctx.enter_context(tc.tile_pool(name="ps_h", bufs=2, space="PSUM"))
        ps_lg = ctx.enter_context(tc.tile_pool(name="ps_lg", bufs=1, space="PSUM"))
        ps_vw = ctx.enter_context(tc.tile_pool(name="ps_vw", bufs=1, space="PSUM"))
        ps_low = ctx.enter_context(tc.tile_pool(name="ps_low", bufs=1, space="PSUM"))
        ps_out = ctx.enter_context(tc.tile_pool(name="ps_out", bufs=3, space="PSUM"))

        # W1 first on the sync ring (same ring as xt -> FIFO: w1, xt0c0, ...)
        w1_sb = const.tile([128, KD, H], F16)
        nc.sync.dma_start(w1_sb[:], w1_d.ap().rearrange("p (k h) -> p k h", h=H))
        # A first on gpsimd ring (needed right after mm1 starts), then B, smalls
        a_sb = const.tile([128, KD, M], F16)
        nc.gpsimd.dma_start(a_sb[:], a_d.ap().rearrange("p (k m) -> p k m", m=M))
        bb_sb = const.tile([M, D], F16)
        nc.gpsimd.dma_start(bb_sb[:], b_d.ap())
        sm32_sb = const.tile([128, KH + NCH * E], FP)
        nc.gpsimd.dma_start(sm32_sb[:], sm32_d.ap())
        sm16_sb = const.tile([128, KH * E + M + 128], F16)
        nc.gpsimd.dma_start(sm16_sb[:], sm16_d.ap())

        b1_sb = sm32_sb[:, 0:KH]
        b2b_sb = sm32_sb[:, KH : KH + NCH * E].rearrange("p (c e) -> p c e", e=E)
        w2_sb = sm16_sb[:, 0 : KH * E].rearrange("p (k e) -> p k e", e=E)
        e80_sb = sm16_sb[0:E, KH * E : KH * E + M]
        ident = sm16_sb[:, KH * E + M :]

        def emit_loads(t):
            xt_sb = xt_pool.tile([128, KD, TT], F16, name="xt_sb")
            for g in range(XC):
                nc.sync.dma_start(
                    xt_sb[:, g * KC : (g + 1) * KC, :],
                    xt_d.ap()[
                        t * 128 : (t + 1) * 128,
                        g * KC * TT : (g + 1) * KC * TT,
                    ].rearrange("p (k j) -> p k j", j=TT),
                )
            base_sb = base_pool.tile([128, NCH, D], F16, name="base_sb")
            nc.scalar.dma_start(
                base_sb[:],
                base_d.ap()[t * 128 : (t + 1) * 128, :].rearrange(
                    "p (c d) -> p c d", d=D
                ),
            )
            return xt_sb, base_sb

        def emit_router(t, xt_sb):
            # mm1 + low interleaved, chunk-major so PE starts on chunk 0
            h_ps = [
                ps_h.tile([128, TT], FP, tag="hps", name=f"h_ps{h}")
                for h in range(KH)
            ]
            low_ps = ps_low.tile([M, TT], FP)
            for k in range(KD):
                for h in range(KH):
                    nc.tensor.matmul(
                        h_ps[h][:],
                        w1_sb[:, k, h * 128 : (h + 1) * 128],
                        xt_sb[:, k, :],
                        start=(k == 0),
                        stop=(k == KD - 1),
                    )
                nc.tensor.matmul(
                    low_ps[:],
                    a_sb[:, k, :],
                    xt_sb[:, k, :],
                    start=(k == 0),
                    stop=(k == KD - 1),
                )

            # silu(h + b1) = z * sigmoid(z), written as fp16
            z_sb = zs_pool.tile([128, KH, TT], F16, name="z_sb")
            sg_sb = zs_pool.tile([128, KH, TT], F16, name="sg_sb")
            hs_sb = zs_pool.tile([128, KH, TT], F16, name="hs_sb")
            for h in range(KH):
                nc.scalar.activation(
                    z_sb[:, h, :], h_ps[h][:],
                    mybir.ActivationFunctionType.Identity,
                    bias=b1_sb[:, h : h + 1], scale=1.0,
                )
                nc.scalar.activation(
                    sg_sb[:, h, :], h_ps[h][:],
                    mybir.ActivationFunctionType.Sigmoid,
                    bias=b1_sb[:, h : h + 1], scale=1.0,
                )
                nc.vector.tensor_tensor(
                    hs_sb[:, h, :], z_sb[:, h, :], sg_sb[:, h, :], A.mult
                )

            # logits token-major: lg[tok, e] = sum_h hs[:,h,tokblk]^T @ W2[h]
            lg_ps = ps_lg.tile([128, NCH, E], FP)
            for c in range(NCH):
                for h in range(KH):
                    nc.tensor.matmul(
                        lg_ps[:, c, :],
                        hs_sb[:, h, c * 128 : (c + 1) * 128],
                        w2_sb[:, h, :],
                        start=(h == 0),
                        stop=(h == KH - 1),
                    )

            # top-2 weights: w1 = sigmoid(m1-m2) for argmax, 1-w1 for argmax2
            Ls = sm_pool.tile([128, NCH, E], FP)
            nc.vector.tensor_tensor(Ls[:], lg_ps[:], b2b_sb, A.add)
            nm1 = sm_pool.tile([128, NCH], FP)
            nc.vector.tensor_reduce(
                nm1[:], Ls[:], axis=mybir.AxisListType.X, op=A.max, negate=True
            )
            eq = sm_pool.tile([128, NCH, E], FP)
            mk = sm_pool.tile([128, NCH, E], FP)
            for c in range(NCH):
                nc.vector.tensor_scalar(
                    eq[:, c, :], Ls[:, c, :], nm1[:, c : c + 1], 0.0,
                    op0=A.add, op1=A.is_equal,
                )
                nc.vector.scalar_tensor_tensor(
                    mk[:, c, :], eq[:, c, :], NEG_BIG, Ls[:, c, :],
                    op0=A.mult, op1=A.add,
                )
            nm2 = sm_pool.tile([128, NCH], FP)
            nc.vector.tensor_reduce(
                nm2[:], mk[:], axis=mybir.AxisListType.X, op=A.max, negate=True
            )
            delta = sm_pool.tile([128, NCH], FP)
            nc.vector.tensor_tensor(delta[:], nm2[:], nm1[:], A.subtract)
            s_sg = sm_pool.tile([128, NCH], FP)
            nc.scalar.activation(
                s_sg[:], delta[:], mybir.ActivationFunctionType.Sigmoid
            )
            s1m = sm_pool.tile([128, NCH], FP)
            nc.vector.tensor_scalar(
                s1m[:], s_sg[:], -1.0, 1.0, op0=A.mult, op1=A.add
            )
            s2m = sm_pool.tile([128, NCH], FP)
            nc.vector.tensor_scalar(
                s2m[:], s_sg[:], 2.0, -1.0, op0=A.mult, op1=A.add
            )
            ge2 = sm_pool.tile([128, NCH, E], FP)
            tmp = sm_pool.tile([128, NCH, E], FP)
            v = sm_pool.tile([128, NCH, E], F16)
            for c in range(NCH):
                nc.vector.tensor_scalar(
                    ge2[:, c, :], Ls[:, c, :], nm2[:, c : c + 1], 0.0,
                    op0=A.add, op1=A.is_ge,
                )
                nc.vector.tensor_scalar(
                    tmp[:, c, :], eq[:, c, :], s2m[:, c : c + 1], None,
                    op0=A.mult,
                )
                nc.vector.scalar_tensor_tensor(
                    v[:, c, :], ge2[:, c, :], s1m[:, c : c + 1], tmp[:, c, :],
                    op0=A.mult, op1=A.add,
                )

            # expand weights to stacked expert-rank dim: vT [E,TT] -> [M,TT]
            vt_ps = ps_vw.tile([E, TT], F16, tag="vw", name="vt_ps")
            for c in range(NCH):
                nc.tensor.transpose(
                    vt_ps[:, c * 128 : (c + 1) * 128], v[:, c, :], ident
                )
            vt_sb = sm_pool.tile([E, TT], F16)
            nc.scalar.copy(vt_sb[:], vt_ps[:])
            we_ps = ps_vw.tile([M, TT], FP, tag="vw", name="we_ps")
            nc.tensor.matmul(we_ps[:], e80_sb, vt_sb[:], start=True, stop=True)
            we_sb = lsc_pool.tile([M, TT], F16, name="we_sb")
            nc.scalar.copy(we_sb[:], we_ps[:])

            lsc_sb = lsc_pool.tile([M, TT], F16, name="lsc_sb")
            nc.vector.tensor_tensor(lsc_sb[:], low_ps[:], we_sb[:], A.mult)
            return lsc_sb

        def emit_finals(t, lsc_sb, base_sb):
            # out[tok, :] = (lsc^T @ B_all) + base, stored fp16 per 128-token
            # chunk; epilogue alternates DVE-direct and ACT-copy + Pool-add.
            for c in range(NCH):
                o_sb = out_pool.tile([128, D], F16, name="o_sb")
                for db in range(ND):
                    o_ps = ps_out.tile([128, 512], FP)
                    nc.tensor.matmul(
                        o_ps[:],
                        lsc_sb[:, c * 128 : (c + 1) * 128],
                        bb_sb[:, db * 512 : (db + 1) * 512],
                        start=True, stop=True,
                    )
                    if c % 2 == 0:
                        nc.vector.tensor_tensor(
                            o_sb[:, db * 512 : (db + 1) * 512],
                            o_ps[:],
                            base_sb[:, c, db * 512 : (db + 1) * 512],
                            A.add,
                        )
                    else:
                        cmb_sb = cmb_pool.tile([128, 512], F16, name="cmb_sb")
                        nc.scalar.copy(cmb_sb[:], o_ps[:])
                        nc.gpsimd.tensor_tensor(
                            o_sb[:, db * 512 : (db + 1) * 512],
                            cmb_sb[:],
                            base_sb[:, c, db * 512 : (db + 1) * 512],
                            A.add,
                        )
                nc.gpsimd.dma_start(
                    out_d.ap()[t * 128 : (t + 1) * 128, c * D : (c + 1) * D],
                    o_sb[:],
                )

        cur = emit_loads(0)
        pending = None
        for t in range(NT):
            if pending is not None:
                emit_finals(*pending)
            nxt = emit_loads(t + 1) if t + 1 < NT else None
            lsc_sb = emit_router(t, cur[0])
            pending = (t, lsc_sb, cur[1])
            cur = nxt
        emit_finals(*pending)

    nc.compile()
    return nc


def _host_prep(x, base_output, A, B, W1, b1, W2, b2, n_cores=N_CORES, TT=TT,
               scaling=SCALING):
    Bb, S_, Dd = x.shape
    E_, _, R_ = A.shape
    N = Bb * S_
    TOKc = N // n_cores
    NCH = TT // 128
    KD = Dd // 128
    KH = W1.shape[1] // 128
    NT = TOKc // TT
    M = E_ * R_
    xf = np.asarray(x, np.float32).reshape(N, Dd).astype(np.float16)
    bf = np.asarray(base_output, np.float32).reshape(N, Dd).astype(np.float16)
    a_all = A.transpose(1, 0, 2).reshape(Dd, M)
    a_all = np.ascontiguousarray(
        a_all.reshape(KD, 128, M).transpose(1, 0, 2).reshape(128, -1),
        np.float16)
    b_all = np.ascontiguousarray(B.reshape(M, Dd) * scaling, np.float16)
    # packed f32 smalls: b1 [128, KH] | b2 broadcast [128, NCH*E]
    b1v = np.asarray(b1, np.float32).reshape(KH, 128).T
    b2b = np.broadcast_to(
        np.tile(np.asarray(b2, np.float32), NCH)[None, :], (128, NCH * E_)
    )
    sm32 = np.ascontiguousarray(np.concatenate([b1v, b2b], axis=1), np.float32)
    # packed f16 smalls: w2 [128, KH*E] | e80 [128, M] | ident [128, 128]
    w2p = (np.asarray(W2, np.float32)
           .reshape(KH, 128, E_).transpose(1, 0, 2).reshape(128, KH * E_))
    e80 = np.zeros((128, M), np.float32)
    for e in range(E_):
        e80[e, e * R_ : (e + 1) * R_] = 1.0
    ident = np.eye(128, dtype=np.float32)
    sm16 = np.ascontiguousarray(
        np.concatenate([w2p, e80, ident], axis=1), np.float16
    )
    shared = {
        "a_all": a_all,
        "b_all": b_all,
        "w1": np.ascontiguousarray(
            np.asarray(W1, np.float32).reshape(KD, 128, -1)
            .transpose(1, 0, 2).reshape(128, -1)).astype(np.float16),
        "sm32": sm32,
        "sm16": sm16,
    }
    in_maps = []
    for i in range(n_cores):
        m = dict(shared)
        xc = xf[i * TOKc : (i + 1) * TOKc]
        bc = bf[i * TOKc : (i + 1) * TOKc]
        m["xt"] = np.ascontiguousarray(
            xc.reshape(NT, TT, KD, 128).transpose(0, 3, 2, 1)
            .reshape(NT * 128, KD * TT))
        m["base"] = np.ascontiguousarray(
            bc.reshape(NT, NCH, 128, Dd).transpose(0, 2, 1, 3)
            .reshape(NT * 128, NCH * Dd))
        in_maps.append(m)
    return in_maps, (N, TOKc, Dd, NT, NCH)


_NC_CACHE = {}


def _get_nc():
    if "nc" not in _NC_CACHE:
        _NC_CACHE["nc"] = _build_nc()
    return _NC_CACHE["nc"]


def kernel(x, base_output, A, B, W1, b1, W2, b2, _trace=False):
    x = np.asarray(x)
    base_output = np.asarray(base_output)
    nc = _get_nc()
    in_maps, (N, TOKc, Dd, NT, NCH) = _host_prep(
        np.asarray(x, np.float32), np.asarray(base_output, np.float32),
        np.asarray(A, np.float32), np.asarray(B, np.float32),
        np.asarray(W1, np.float32), np.asarray(b1, np.float32),
        np.asarray(W2, np.float32), np.asarray(b2, np.float32),
    )
    res = run_bass_kernel_spmd(
        nc, in_maps, core_ids=list(range(N_CORES)), trace=_trace
    )
    outs = []
    for i in range(N_CORES):
        o = res.results[i]["out"]
        o = o.reshape(NT, 128, NCH, Dd).transpose(0, 2, 1, 3).reshape(TOKc, Dd)
        outs.append(o)
    out = np.concatenate(outs, axis=0).astype(np.float32)
    out = out.reshape(x.shape)
    if _trace:
        kernel._last_exec_time_ns = res.exec_time_ns
        kernel._last_results = res
    return out


# revision 9
# speedup vs baseline: 1.2470x; 1.2470x over previous
"""MoLoRA (top-2 MoE LoRA routing) Trainium2 kernel — fp16 data-path version.

Full inputs -> shard tokens across 8 NeuronCores -> Bass/Tile kernel per core
-> gather full output.

Math (per token):
  logits = silu(x @ W1 + b1) @ W2 + b2
  top-2 renormalized softmax weights: w_top1 = sigmoid(l1 - l2),
  w_top2 = 1 - w_top1 (exact identity — no exp/renorm needed).
  combined = sum_e w_e * (x @ A_e @ B_e) * 2.0 ; out = base + combined.

Kernel strategy per core (2048 tokens, fp16 on the wire):
  - x is pre-transposed on the HOST into xT tiles [128 D-part, KD, TT] so no
    on-chip transposes are needed; contractions over D run at full PE rate.
  -

    Variable tile sizes (256,256,512,512,512 tokens): small leading tiles get
    compute and output stores started early so HBM stays saturated; DMA rings
    are specialized (sync: A,W1,xT loads; scalar: B,base loads; gpsimd:
    packed small constants + output stores).
  - Router mm1 in token-on-free layout; logits are produced directly
    token-major by a second matmul with hs chunks as the stationary operand.
  - Top-2 weights via max / masked-second-max / sigmoid(delta).
  - Selected-expert weights expand to the stacked expert-rank dim [80] with a
    0/1 matmul; lowT = A_all^T @ xT is scaled by them; combined output is
    lsc^T @ B_all (B pre-scaled by 2.0 on host) + base, written as fp16 and
    widened to fp32 on the host. The +base epilogue alternates DVE-direct
    adds with ACT-copy + DVE-fast-add to balance engine load.
"""
import sys

for _p in ("/opt/trn_rl_repo",):
    if _p not in sys.path:
        sys.path.insert(0, _p)

import numpy as np
from contextlib import ExitStack

import concourse.bass as bass
import concourse.tile as tile
from concourse import bacc, mybir
from concourse.bass_utils import run_bass_kernel_spmd

FP = mybir.dt.float32
F16 = mybir.dt.float16
NEG_BIG = -1e30

N_CORES = 8
B_, S, D = 4, 4096, 2048
E, R, H = 5, 16, 256
SCALING = 32.0 / 16.0
TOK = (B_ * S) // N_CORES
TTS = (256, 256, 512, 512, 512)
assert sum(TTS) == TOK


def _build_nc(TOK=TOK, D=D, H=H, E=E, R=R, n_cores=N_CORES):
    from concourse.alu_op_type import AluOpType as A

    KD = D // 128
    KH = H // 128
    M = E * R
    ND = D // 512
    TTMAX = max(TTS)

    nc = bacc.Bacc("TRN2", num_devices=n_cores, debug=False)

    xt_d = nc.dram_tensor("xt", [TOK * D], F16, kind="ExternalInput")
    base_d = nc.dram_tensor("base", [TOK * D], F16, kind="ExternalInput")
    a_d = nc.dram_tensor("a_all", [128, KD * M], F16, kind="ExternalInput")
    b_d = nc.dram_tensor("b_all", [M, D], F16, kind="ExternalInput")
    w1_d = nc.dram_tensor("w1", [128, KD * H], F16, kind="ExternalInput")
    # packed small constants: f32 [128, KH + maxNCH*E] = b1 | b2-broadcast
    NCHM = TTMAX // 128
    sm32_d = nc.dram_tensor("sm32", [128, KH + NCHM * E], FP, kind="ExternalInput")
    # packed small constants: f16 [128, KH*E + M + 128] = w2 | e80 | ident
    sm16_d = nc.dram_tensor(
        "sm16", [128, KH * E + M + 128], F16, kind="ExternalInput"
    )
    out_d = nc.dram_tensor("out", [TOK * D], F16, kind="ExternalOutput")

    with tile.TileContext(nc) as tc, ExitStack() as ctx:
        const = ctx.enter_context(tc.tile_pool(name="const", bufs=1))
        xt_pool = ctx.enter_context(tc.tile_pool(name="xt", bufs=3))
        base_pool = ctx.enter_context(tc.tile_pool(name="basep", bufs=3))
        out_pool = ctx.enter_context(tc.tile_pool(name="outp", bufs=6))
        cmb_pool = ctx.enter_context(tc.tile_pool(name="cmb", bufs=4))
        zs_pool = ctx.enter_context(tc.tile_pool(name="zs", bufs=2))
        sm_pool = ctx.enter_context(tc.tile_pool(name="sm", bufs=2))
        lsc_pool = ctx.enter_context(tc.tile_pool(name="lsc", bufs=2))

        ps_h = ctx.enter_context(tc.tile_pool(name="ps_h", bufs=2, space="PSUM"))
        ps_lg = ctx.enter_context(tc.tile_pool(name="ps_lg", bufs=1, space="PSUM"))
        ps_vw = ctx.enter_context(tc.tile_pool(name="ps_vw", bufs=1, space="PSUM"))
        ps_low = ctx.enter_context(tc.tile_pool(name="ps_low", bufs=1, space="PSUM"))
        ps_out = ctx.enter_context(tc.tile_pool(name="ps_out", bufs=3, space="PSUM"))

        # sync ring order: A (small, needed by interleaved low-mm), W1, xT...
        a_sb = const.tile([128, KD, M], F16)
        nc.sync.dma_start(a_sb[:], a_d.ap().rearrange("p (k m) -> p k m", m=M))
        w1_sb = const.tile([128, KD, H], F16)
        nc.sync.dma_start(w1_sb[:], w1_d.ap().rearrange("p (k h) -> p k h", h=H))
        # scalar ring: B then base tiles
        bb_sb = const.tile([M, D], F16)
        nc.scalar.dma_start(bb_sb[:], b_d.ap())
        # gpsimd ring: packed smalls, then output stores
        sm32_sb = const.tile([128, KH + NCHM * E], FP)
        nc.gpsimd.dma_start(sm32_sb[:], sm32_d.ap())
        sm16_sb = const.tile([128, KH * E + M + 128], F16)
        nc.gpsimd.dma_start(sm16_sb[:], sm16_d.ap())

        b1_sb = sm32_sb[:, 0:KH]
        b2b_full = sm32_sb[:, KH : KH + NCHM * E]
        w2_sb = sm16_sb[:, 0 : KH * E].rearrange("p (k e) -> p k e", e=E)
        e80_sb = sm16_sb[0:E, KH * E : KH * E + M]
        ident = sm16_sb[:, KH * E + M :]

        def emit_loads(t, off, tt):
            nch = tt // 128
            xt_sb = xt_pool.tile([128, KD, tt], F16, tag="xt_sb", name="xt_sb")
            nc.sync.dma_start(
                xt_sb[:],
                xt_d.ap()[off * D : (off + tt) * D].rearrange(
                    "(p k j) -> p k j", p=128, k=KD
                ),
            )
            base_sb = base_pool.tile(
                [128, nch, D], F16, tag="base_sb", name="base_sb"
            )
            nc.scalar.dma_start(
                base_sb[:],
                base_d.ap()[off * D : (off + tt) * D].rearrange(
                    "(p c d) -> p c d", p=128, c=nch
                ),
            )
            return xt_sb, base_sb

        def emit_router(t, tt, xt_sb):
            nch = tt // 128
            # mm1 + low interleaved k-major (A lands before W1 on sync ring)
            h_ps = [
                ps_h.tile([128, tt], FP, tag="hps", name=f"h_ps{h}")
                for h in range(KH)
            ]
            low_ps = ps_low.tile([M, tt], FP, tag="low")
            for k in range(KD):
                for h in range(KH):
                    nc.tensor.matmul(
                        h_ps[h][:],
                        w1_sb[:, k, h * 128 : (h + 1) * 128],
                        xt_sb[:, k, :],
                        start=(k == 0),
                        stop=(k == KD - 1),
                    )
                nc.tensor.matmul(
                    low_ps[:],
                    a_sb[:, k, :],
                    xt_sb[:, k, :],
                    start=(k == 0),
                    stop=(k == KD - 1),
                )

            # silu(h + b1) = z * sigmoid(z), written as fp16
            z_sb = zs_pool.tile([128, KH, tt], F16, tag="z", name="z_sb")
            sg_sb = zs_pool.tile([128, KH, tt], F16, tag="sg", name="sg_sb")
            hs_sb = zs_pool.tile([128, KH, tt], F16, tag="hs", name="hs_sb")
            for h in range(KH):
                nc.scalar.activation(
                    z_sb[:, h, :], h_ps[h][:],
                    mybir.ActivationFunctionType.Identity,
                    bias=b1_sb[:, h : h + 1], scale=1.0,
                )
                nc.scalar.activation(
                    sg_sb[:, h, :], h_ps[h][:],
                    mybir.ActivationFunctionType.Sigmoid,
                    bias=b1_sb[:, h : h + 1], scale=1.0,
                )
                nc.vector.tensor_tensor(
                    hs_sb[:, h, :], z_sb[:, h, :], sg_sb[:, h, :], A.mult
                )

            # logits token-major: lg[tok, e] = sum_h hs[:,h,tokblk]^T @ W2[h]
            lg_ps = ps_lg.tile([128, nch, E], FP, tag="lg")
            for c in range(nch):
                for h in range(KH):
                    nc.tensor.matmul(
                        lg_ps[:, c, :],
                        hs_sb[:, h, c * 128 : (c + 1) * 128],
                        w2_sb[:, h, :],
                        start=(h == 0),
                        stop=(h == KH - 1),
                    )

            # top-2 weights: w1 = sigmoid(m1-m2) for argmax, 1-w1 for argmax2
            b2b_sb = b2b_full[:, 0 : nch * E].rearrange("p (c e) -> p c e", e=E)
            Ls = sm_pool.tile([128, nch, E], FP, tag="Ls")
            nc.vector.tensor_tensor(Ls[:], lg_ps[:], b2b_sb, A.add)
            nm1 = sm_pool.tile([128, nch], FP, tag="nm1")
            nc.vector.tensor_reduce(
                nm1[:], Ls[:], axis=mybir.AxisListType.X, op=A.max, negate=True
            )
            eq = sm_pool.tile([128, nch, E], FP, tag="eq")
            mk = sm_pool.tile([128, nch, E], FP, tag="mk")
            for c in range(nch):
                nc.vector.tensor_scalar(
                    eq[:, c, :], Ls[:, c, :], nm1[:, c : c + 1], 0.0,
                    op0=A.add, op1=A.is_equal,
                )
                nc.vector.scalar_tensor_tensor(
                    mk[:, c, :], eq[:, c, :], NEG_BIG, Ls[:, c, :],
                    op0=A.mult, op1=A.add,
                )
            nm2 = sm_pool.tile([128, nch], FP, tag="nm2")
            nc.vector.tensor_reduce(
                nm2[:], mk[:], axis=mybir.AxisListType.X, op=A.max, negate=True
            )
            delta = sm_pool.tile([128, nch], FP, tag="delta")
            nc.vector.tensor_tensor(delta[:], nm2[:], nm1[:], A.subtract)
            s_sg = sm_pool.tile([128, nch], FP, tag="s_sg")
            nc.scalar.activation(
                s_sg[:], delta[:], mybir.ActivationFunctionType.Sigmoid
            )
            s1m = sm_pool.tile([128, nch], FP, tag="s1m")
            nc.vector.tensor_scalar(
                s1m[:], s_sg[:], -1.0, 1.0, op0=A.mult, op1=A.add
            )
            s2m = sm_pool.tile([128, nch], FP, tag="s2m")
            nc.vector.tensor_scalar(
                s2m[:], s_sg[:], 2.0, -1.0, op0=A.mult, op1=A.add
            )
            ge2 = sm_pool.tile([128, nch, E], FP, tag="ge2")
            tmp = sm_pool.tile([128, nch, E], FP, tag="tmp")
            v = sm_pool.tile([128, nch, E], F16, tag="v")
            for c in range(nch):
                nc.vector.tensor_scalar(
                    ge2[:, c, :], Ls[:, c, :], nm2[:, c : c + 1], 0.0,
                    op0=A.add, op1=A.is_ge,
                )
                nc.vector.tensor_scalar(
                    tmp[:, c, :], eq[:, c, :], s2m[:, c : c + 1], None,
                    op0=A.mult,
                )
                nc.vector.scalar_tensor_tensor(
                    v[:, c, :], ge2[:, c, :], s1m[:, c : c + 1], tmp[:, c, :],
                    op0=A.mult, op1=A.add,
                )

            # expand weights to stacked expert-rank dim: vT [E,tt] -> [M,tt]
            vt_ps = ps_vw.tile([E, tt], F16, tag="vw", name="vt_ps")
            for c in range(nch):
                nc.tensor.transpose(
                    vt_ps[:, c * 128 : (c + 1) * 128], v[:, c, :], ident
                )
            vt_sb = sm_pool.tile([E, tt], F16, tag="vt")
            nc.scalar.copy(vt_sb[:], vt_ps[:])
            we_ps = ps_vw.tile([M, tt], FP, tag="vw", name="we_ps")
            nc.tensor.matmul(we_ps[:], e80_sb, vt_sb[:], start=True, stop=True)
            we_sb = lsc_pool.tile([M, tt], F16, tag="we", name="we_sb")
            nc.scalar.copy(we_sb[:], we_ps[:])

            lsc_sb = lsc_pool.tile([M, tt], F16, tag="lsc", name="lsc_sb")
            nc.vector.tensor_tensor(lsc_sb[:], low_ps[:], we_sb[:], A.mult)
            return lsc_sb

        def emit_finals(t, off, tt, lsc_sb, base_sb):
            # out[tok, :] = (lsc^T @ B_all) + base, stored fp16 per 128-token
            # chunk; epilogue alternates DVE-direct and ACT-copy + DVE-add.
            nch = tt // 128
            for c in range(nch):
                o_sb = out_pool.tile([128, D], F16, tag="o_sb", name="o_sb")
                for db in range(ND):
                    o_ps = ps_out.tile([128, 512], FP, tag="o_ps")
                    nc.tensor.matmul(
                        o_ps[:],
                        lsc_sb[:, c * 128 : (c + 1) * 128],
                        bb_sb[:, db * 512 : (db + 1) * 512],
                        start=True, stop=True,
                    )
                    if (c + db) % 2 == 0:
                        nc.vector.tensor_tensor(
                            o_sb[:, db * 512 : (db + 1) * 512],
                            o_ps[:],
                            base_sb[:, c, db * 512 : (db + 1) * 512],
                            A.add,
                        )
                    else:
                        cmb_sb = cmb_pool.tile(
                            [128, 512], F16, tag="cmb", name="cmb_sb"
                        )
                        nc.scalar.copy(cmb_sb[:], o_ps[:])
                        nc.vector.tensor_tensor(
                            o_sb[:, db * 512 : (db + 1) * 512],
                            cmb_sb[:],
                            base_sb[:, c, db * 512 : (db + 1) * 512],
                            A.add,
                        )
                nc.gpsimd.dma_start(
                    out_d.ap()[
                        (off + c * 128) * D : (off + (c + 1) * 128) * D
                    ].rearrange("(p d) -> p d", p=128),
                    o_sb[:],
                )

        offs = [sum(TTS[:i]) for i in range(len(TTS))]
        cur = emit_loads(0, offs[0], TTS[0])
        pending = None
        for t in range(len(TTS)):
            if pending is not None:
                emit_finals(*pending)
            nxt = (
                emit_loads(t + 1, offs[t + 1], TTS[t + 1])
                if t + 1 < len(TTS)
                else None
            )
            lsc_sb = emit_router(t, TTS[t], cur[0])
            pending = (t, offs[t], TTS[t], lsc_sb, cur[1])
            cur = nxt
        emit_finals(*pending)

    nc.compile()
    return nc


def _host_prep(x, base_output, A, B, W1, b1, W2, b2, n_cores=N_CORES,
               scaling=SCALING):
    Bb, S_, Dd = x.shape
    E_, _, R_ = A.shape
    N = Bb * S_
    TOKc = N // n_cores
    KD = Dd // 128
    KH = W1.shape[1] // 128
    M = E_ * R_
    NCHM = max(TTS) // 128
    xf = np.asarray(x, np.float32).reshape(N, Dd).astype(np.float16)
    bf = np.asarray(base_output, np.float32).reshape(N, Dd).astype(np.float16)
    a_all = A.transpose(1, 0, 2).reshape(Dd, M)
    a_all = np.ascontiguousarray(
        a_all.reshape(KD, 128, M).transpose(1, 0, 2).reshape(128, -1),
        np.float16)
    b_all = np.ascontiguousarray(B.reshape(M, Dd) * scaling, np.float16)
    b1v = np.asarray(b1, np.float32).reshape(KH, 128).T
    b2b = np.broadcast_to(
        np.tile(np.asarray(b2, np.float32), NCHM)[None, :], (128, NCHM * E_)
    )
    sm32 = np.ascontiguousarray(np.concatenate([b1v, b2b], axis=1), np.float32)
    w2p = (np.asarray(W2, np.float32)
           .reshape(KH, 128, E_).transpose(1, 0, 2).reshape(128, KH * E_))
    e80 = np.zeros((128, M), np.float32)
    for e in range(E_):
        e80[e, e * R_ : (e + 1) * R_] = 1.0
    ident = np.eye(128, dtype=np.float32)
    sm16 = np.ascontiguousarray(
        np.concatenate([w2p, e80, ident], axis=1), np.float16
    )
    shared = {
        "a_all": a_all,
        "b_all": b_all,
        "w1": np.ascontiguousarray(
            np.asarray(W1, np.float32).reshape(KD, 128, -1)
            .transpose(1, 0, 2).reshape(128, -1)).astype(np.float16),
        "sm32": sm32,
        "sm16": sm16,
    }
    offs = [sum(TTS[:i]) for i in range(len(TTS))]
    in_maps = []
    for i in range(n_cores):
        m = dict(shared)
        xc = xf[i * TOKc : (i + 1) * TOKc]
        bc = bf[i * TOKc : (i + 1) * TOKc]
        xts, bts = [], []
        for off, tt in zip(offs, TTS):
            xts.append(
                xc[off : off + tt].reshape(tt, KD, 128)
                .transpose(2, 1, 0).reshape(-1)
            )
            bts.append(
                bc[off : off + tt].reshape(tt // 128, 128, Dd)
                .transpose(1, 0, 2).reshape(-1)
            )
        m["xt"] = np.ascontiguousarray(np.concatenate(xts))
        m["base"] = np.ascontiguousarray(np.concatenate(bts))
        in_maps.append(m)
    return in_maps, (N, TOKc, Dd)


_NC_CACHE = {}


def _get_nc():
    if "nc" not in _NC_CACHE:
        _NC_CACHE["nc"] = _build_nc()
    return _NC_CACHE["nc"]


def kernel(x, base_output, A, B, W1, b1, W2, b2, _trace=False):
    x = np.asarray(x)
    base_output = np.asarray(base_output)
    nc = _get_nc()
    in_maps, (N, TOKc, Dd) = _host_prep(
        np.asarray(x, np.float32), np.asarray(base_output, np.float32),
        np.asarray(A, np.float32), np.asarray(B, np.float32),
        np.asarray(W1, np.float32), np.asarray(b1, np.float32),
        np.asarray(W2, np.float32), np.asarray(b2, np.float32),
    )
    res = run_bass_kernel_spmd(
        nc, in_maps, core_ids=list(range(N_CORES)), trace=_trace
    )
    outs = []
    for i in range(N_CORES):
        # stores are plain token-major: row = off + c*128 + p
        oc = res.results[i]["out"].reshape(TOKc, Dd)
        outs.append(oc)
    out = np.concatenate(outs, axis=0).astype(np.float32)
    out = out.reshape(x.shape)
    if _trace:
        kernel._last_exec_time_ns = res.exec_time_ns
        kernel._last_results = res
    return out
